# revision 1
# baseline (speedup 1.0000x reference)
"""Trainium2 Bass kernel: causal multi-head attention with interleaved RoPE.

Problem shapes (hardcoded): x [2, 2048, 1024], 16 heads of dk=64.
Sharding: 8 cores = 2 batches x 4 head-groups (4 heads each). Each core
computes its head-slice Q/K/V projections, RoPE, causal attention, and a
partial output through its Wo row-slice; the host sums the 4 partials per
batch and adds bo.

RoPE trick: attention scores are invariant to any permutation of the dk
axis applied to both Q and K, so the Wq/Wk columns are permuted on the host
into a "quadrant half-split" layout where each rotation pair partner sits
exactly 16 partitions away inside the same 32-partition quadrant. The DVE
stream_shuffle (a per-quadrant 32-way permute) then produces the swapped
operand, and RoPE becomes: rot = q * cosT + shuffle(q) * sinT with
host-precomputed tables (sinT carries the sign).
"""

import os
from contextlib import ExitStack

import numpy as np

import concourse.bass as bass
import concourse.mybir as mybir
import concourse.tile as tile

B, S, D, H = 2, 2048, 1024, 16
DK = D // H  # 64
HG = 4  # heads per core
NCOLS = HG * DK  # 256 columns of the projection per core
THETA = 10000.0
SCALE = 1.0 / float(np.sqrt(DK))
N_CORES = 8

F32 = mybir.dt.float32
F32R = mybir.dt.float32r

# matmul operand dtype: float32r (= TF32, 10-bit mantissa) streams 1 col/cycle
# on the PE vs 4 for float32. Operands must be *rounded* to TF32: DMA-fed
# tensors are pre-rounded on the host and declared float32r; on-chip operand
# producers write float32r directly. Numerics validated in test.py.
USE_F32R = os.environ.get("KERNEL_F32", "0") != "1"
MMDT = F32R if USE_F32R else F32


def round_tf32(a):
    """Round fp32 array to TF32 (RNE to 10-bit mantissa)."""
    if not USE_F32R:
        return np.ascontiguousarray(a, dtype=np.float32)
    u = np.ascontiguousarray(a, dtype=np.float32).view(np.uint32).copy()
    u += 0x0FFF + ((u >> 13) & 1)
    u &= np.uint32(0xFFFFE000)
    return u.view(np.float32)


# ---------------------------------------------------------------------------
# host-side prep
# ---------------------------------------------------------------------------

def _rope_perm():
    """Within-head column permutation pi: new row r -> original dk index."""
    perm = np.empty(DK, dtype=np.int64)
    for r in range(DK):
        q, m = divmod(r, 32)
        if m < 16:
            perm[r] = 2 * (16 * q + m)
        else:
            perm[r] = 2 * (16 * q + m - 16) + 1
    return perm


_PERM = _rope_perm()
SHUF_MASK = list(range(16, 32)) + list(range(16))  # swap 16-halves per quadrant


def _rope_tables(pos):
    """cosT/sinT [128, S] fp32 for the permuted layout. pos: [S] int."""
    inv_freq = (np.float32(THETA) ** (-(np.arange(0, DK, 2, dtype=np.float32) / np.float32(DK))))  # [32]
    ang = pos.astype(np.float32)[:, None] * inv_freq[None, :]  # [S, 32]
    cos = np.cos(ang)  # [S, 32]
    sin = np.sin(ang)
    cosT = np.empty((128, S), dtype=np.float32)
    sinT = np.empty((128, S), dtype=np.float32)
    for p in range(128):
        r = p % DK
        q, m = divmod(r, 32)
        if m < 16:
            i = 16 * q + m
            sgn = -1.0
        else:
            i = 16 * q + m - 16
            sgn = 1.0
        cosT[p] = cos[:, i]
        sinT[p] = np.float32(sgn) * sin[:, i]
    return cosT, sinT


def make_core_inputs(x, token_position, Wq, bq, Wk, bk, Wv, bv, Wo, bo):
    """Build the 8 per-core input maps."""
    x = np.asarray(x, dtype=np.float32)
    token_position = np.asarray(token_position)
    Wq, Wk, Wv, Wo = (np.asarray(w, dtype=np.float32) for w in (Wq, Wk, Wv, Wo))
    bq, bk, bv = (np.asarray(b_, dtype=np.float32) for b_ in (bq, bk, bv))

    in_maps = []
    tables = {}
    for c in range(N_CORES):
        b, hg = divmod(c, HG)
        heads = range(HG * hg, HG * hg + HG)
        # permuted q/k column indices for this core's heads
        cols_qk = np.concatenate([DK * h + _PERM for h in heads])
        cols_v = np.arange(NCOLS * hg, NCOLS * hg + NCOLS)
        if b not in tables:
            tables[b] = _rope_tables(np.asarray(token_position[b]))
        cosT, sinT = tables[b]
        wo_rows = Wo[cols_v, :]  # [256, 1024]
        in_maps.append({
            "xT": round_tf32(x[b].T),                               # [1024, 2048]
            "wq": round_tf32(Wq[:, cols_qk]),                       # [1024, 256]
            "wk": round_tf32(Wk[:, cols_qk]),
            "wv": round_tf32(Wv[:, cols_v]),
            "wo": round_tf32(wo_rows.reshape(HG, DK, D).transpose(1, 0, 2)),  # [64, 4, 1024]
            "bq": round_tf32(bq[cols_qk][None, :]),                 # [1, 256]
            "bk": round_tf32(bk[cols_qk][None, :]),
            "bv": round_tf32(bv[cols_v][None, :]),
            "ones_row": round_tf32(np.ones((1, 512), np.float32)),
            "onesc": round_tf32(np.ones((128, 64), np.float32)),
            "cosT": cosT,
            "sinT": sinT,
        })
    return in_maps


# ---------------------------------------------------------------------------
# device program
# ---------------------------------------------------------------------------

def build_program(with_bias=False):
    from concourse import bacc, library_config
    nc = bacc.Bacc("TRN2", debug=False)

    xT = nc.declare_dram_parameter("xT", [D, S], MMDT, isOutput=False).ap()
    wq = nc.declare_dram_parameter("wq", [D, NCOLS], MMDT, isOutput=False).ap()
    wk = nc.declare_dram_parameter("wk", [D, NCOLS], MMDT, isOutput=False).ap()
    wv = nc.declare_dram_parameter("wv", [D, NCOLS], MMDT, isOutput=False).ap()
    wo = nc.declare_dram_parameter("wo", [DK, HG, D], MMDT, isOutput=False).ap()
    bq = nc.declare_dram_parameter("bq", [1, NCOLS], MMDT, isOutput=False).ap()
    bk = nc.declare_dram_parameter("bk", [1, NCOLS], MMDT, isOutput=False).ap()
    bv = nc.declare_dram_parameter("bv", [1, NCOLS], MMDT, isOutput=False).ap()
    ones_row_d = nc.declare_dram_parameter("ones_row", [1, 512], MMDT, isOutput=False).ap()
    onesc_d = nc.declare_dram_parameter("onesc", [128, DK], MMDT, isOutput=False).ap()
    cosT = nc.declare_dram_parameter("cosT", [128, S], F32, isOutput=False).ap()
    sinT = nc.declare_dram_parameter("sinT", [128, S], F32, isOutput=False).ap()
    out = nc.declare_dram_parameter("out", [S, D], F32, isOutput=True).ap()

    SB = 512            # sq block width
    NSB = S // SB       # 4
    NST = S // 128      # 16 key tiles / V tiles
    NDC = D // 128      # 8 contraction chunks
    GW = 2              # key tiles per score-psum group

    with tile.TileContext(nc) as tc, ExitStack() as ctx:
        nc.gpsimd.load_library(library_config.proxy)
        const = ctx.enter_context(tc.tile_pool(name="const", bufs=1))
        sbig = ctx.enter_context(tc.tile_pool(name="sbig", bufs=1))
        xts = ctx.enter_context(tc.tile_pool(name="xts", bufs=4))
        rtmp = ctx.enter_context(tc.tile_pool(name="rtmp", bufs=2))
        epool = ctx.enter_context(tc.tile_pool(name="epool", bufs=3))
        npool = ctx.enter_context(tc.tile_pool(name="npool", bufs=3))
        opool = ctx.enter_context(tc.tile_pool(name="opool", bufs=2))

        # --- constants / weights resident in SBUF (per-dc tiles: finer deps,
        # so the first projection matmuls start after ~128KB of DMA)
        wq_sb = [const.tile([128, NCOLS], MMDT, tag=f"wq{dc}", name=f"wq{dc}")
                 for dc in range(NDC)]
        wk_sb = [const.tile([128, NCOLS], MMDT, tag=f"wk{dc}", name=f"wk{dc}")
                 for dc in range(NDC)]
        wv_sb = [const.tile([128, NCOLS], MMDT, tag=f"wv{dc}", name=f"wv{dc}")
                 for dc in range(NDC)]
        for dc in range(NDC):
            nc.sync.dma_start(wq_sb[dc][:], wq[128 * dc:128 * dc + 128, :])
            nc.sync.dma_start(wk_sb[dc][:], wk[128 * dc:128 * dc + 128, :])
        cos_sb = const.tile([128, S], F32, tag="cos")
        sin_sb = const.tile([128, S], F32, tag="sin")
        nc.sync.dma_start(cos_sb[:], cosT)
        nc.sync.dma_start(sin_sb[:], sinT)
        for dc in range(NDC):
            nc.sync.dma_start(wv_sb[dc][:], wv[128 * dc:128 * dc + 128, :])
        # wo padded to K=128 with zero rows 64-127: fp32r matmuls with K=64
        # stream at ~2 cycles/row (HW-measured), K=128 at 1 -- zero-padding
        # the contraction nearly halves scores/Wo PE time. DMA'd after the
        # critical-path inputs (only needed in the Wo phase).
        wo_sb = const.tile([128, HG, D], MMDT, tag="wo")
        nc.sync.dma_start(wo_sb[0:DK, :, :], wo)
        for a in range(2):
            nc.vector.tensor_scalar_mul(
                wo_sb[DK:128, 2 * a:2 * a + 2, :],
                sin_sb[DK:128, :].rearrange("p (a b) -> p a b", a=2), 0.0)
        if with_bias:
            bq_sb = const.tile([1, NCOLS], MMDT, tag="bq")
            bk_sb = const.tile([1, NCOLS], MMDT, tag="bk")
            bv_sb = const.tile([1, NCOLS], MMDT, tag="bv")
            nc.sync.dma_start(bq_sb[:], bq)
            nc.sync.dma_start(bk_sb[:], bk)
            nc.sync.dma_start(bv_sb[:], bv)
        ones_row = const.tile([1, SB], MMDT, tag="ones_row")
        nc.sync.dma_start(ones_row[:], ones_row_d)
        onesc_sb = const.tile([128, DK], MMDT, tag="onesc")
        nc.sync.dma_start(onesc_sb[:], onesc_d)

        # Q^T / K^T per (chunk, sq-block): chunk c holds heads {2c, 2c+1}
        qt = [[sbig.tile([128, SB], MMDT, tag=f"qt{c}_{sb}", name=f"qt{c}_{sb}")
               for sb in range(NSB)] for c in range(2)]
        # per-head K^T, zero-padded to 128 partitions (head data on its chunk
        # rows, the complementary 64 rows zeroed)
        kth = [[sbig.tile([128, SB], MMDT, tag=f"kh{h}_{sb}", name=f"kh{h}_{sb}")
                for sb in range(NSB)] for h in range(HG)]
        for h in range(HG):
            zrows = slice(DK, 128) if h % 2 == 0 else slice(0, DK)
            for sb in range(NSB):
                nc.vector.tensor_scalar_mul(kth[h][sb][zrows, :],
                                            cos_sb[zrows, 0:SB], 0.0)
        # V augmented with a ones column per head, per key tile. Head stride
        # padded 65 -> 68 columns so each head's lhsT starts 16B-aligned.
        AUGW = DK + 4
        vaug = [sbig.tile([128, HG * AUGW], MMDT, tag=f"va{st}", name=f"va{st}")
                for st in range(NST)]
        # unnormalized O^T per (head, sq-block), zero-padded to 128 rows
        ot = [[sbig.tile([128, SB], MMDT, tag=f"ot{h}_{j}", name=f"ot{h}_{j}")
               for j in range(NSB)] for h in range(HG)]
        for h in range(HG):
            for j in range(NSB):
                nc.vector.tensor_scalar_mul(ot[h][j][DK:128, :],
                                             cos_sb[DK:128, 0:SB], 0.0)

        # ------------------------------------------------------- projections
        with tc.tile_pool(name="pj_ps", bufs=4, space="PSUM") as pj_ps, \
             tc.tile_pool(name="pv_ps", bufs=4, space="PSUM") as pvp_ps:
            for sb in range(NSB):
                ss = slice(SB * sb, SB * sb + SB)
                xt_t = []
                for dc in range(NDC):
                    t = xts.tile([128, SB], MMDT, tag="xt")
                    nc.sync.dma_start(t[:], xT[128 * dc:128 * dc + 128, ss])
                    xt_t.append(t)
                for c in range(2):
                    ncol = slice(128 * c, 128 * c + 128)
                    for (w_sb, bname) in ((wq_sb, "bq"), (wk_sb, "bk")):
                        ps = pj_ps.tile([128, SB], F32, tag="qk")
                        for dc in range(NDC):
                            nc.tensor.matmul(ps[:], w_sb[dc][:, ncol], xt_t[dc][:],
                                             start=(dc == 0),
                                             stop=(dc == NDC - 1 and not with_bias))
                        if with_bias:
                            b_sb = bq_sb if bname == "bq" else bk_sb
                            nc.tensor.matmul(ps[:], b_sb[0:1, ncol], ones_row[0:1, :],
                                             start=False, stop=True)
                        # rope: dst = ps*cos + shuffle(ps)*sin
                        t_cos = rtmp.tile([128, SB], F32, tag="rc")
                        nc.vector.tensor_mul(t_cos[:], ps[:], cos_sb[:, ss])
                        t_shuf = rtmp.tile([128, SB], F32, tag="rs")
                        nc.vector.stream_shuffle(t_shuf[:], ps[:], SHUF_MASK)
                        t_sin = rtmp.tile([128, SB], F32, tag="rm")
                        nc.gpsimd.tensor_mul(t_sin[:], t_shuf[:], sin_sb[:, ss])
                        if bname == "bq":
                            nc.vector.tensor_add(qt[c][sb][:], t_cos[:], t_sin[:])
                        else:
                            nc.vector.tensor_add(kth[2 * c][sb][0:DK, :],
                                                 t_cos[0:DK, :], t_sin[0:DK, :])
                            nc.vector.tensor_add(kth[2 * c + 1][sb][DK:128, :],
                                                 t_cos[DK:128, :], t_sin[DK:128, :])
                for st4 in range(SB // 128):
                    st = (SB // 128) * sb + st4
                    ps = pvp_ps.tile([128, NCOLS], F32, tag="v")
                    for dc in range(NDC):
                        nc.tensor.matmul(ps[:], xt_t[dc][:, 128 * st4:128 * st4 + 128],
                                         wv_sb[dc][:],
                                         start=(dc == 0),
                                         stop=(dc == NDC - 1 and not with_bias))
                    if with_bias:
                        nc.tensor.matmul(ps[:], ones_row[0:1, 0:128], bv_sb[0:1, :],
                                         start=False, stop=True)
                    # scatter heads into the augmented layout; even heads get
                    # [V | ones], odd heads [ones | V] (so PV psum offset 63
                    # puts their output on partitions 64-127)
                    va = vaug[st][:].rearrange("p (h e) -> p h e", h=HG)
                    nc.vector.tensor_copy(va[:, :, 0:DK],
                                          ps[:].rearrange("p (h k) -> p h k", h=HG))
                    nc.vector.tensor_copy(va[:, :, DK], onesc_sb[:, 0:HG])

        # -------------------------------------------------------- attention
        # S^T layout: psum group = GW key tiles x one sq block; exp on ACT;
        # PV accumulates (V | ones) so row 64 is the softmax denominator.
        with tc.tile_pool(name="sc_ps", bufs=2, space="PSUM") as sc_ps, \
             tc.tile_pool(name="o_ps", bufs=2, space="PSUM") as o_ps, \
             tc.tile_pool(name="bc_ps", bufs=2, space="PSUM") as bc_ps:
            for j in range(NSB):
                sq = slice(SB * j, SB * j + SB)
                for h in range(HG):
                    c, half = divmod(h, 2)
                    rows = slice(DK * half, DK * half + DK)
                    pv = o_ps.tile([128, SB], F32, tag="pv")
                    ngrp = (4 * j + 4) // GW
                    for g in range(ngrp):
                        sc = sc_ps.tile([128, GW * SB], F32, tag="sc")
                        for t in range(GW):
                            i = GW * g + t
                            nc.tensor.matmul(
                                sc[:, SB * t:SB * t + SB],
                                kth[h][i // 4][:, 128 * (i % 4):128 * (i % 4) + 128],
                                qt[c][j][:],
                                start=True, stop=True)
                        e = epool.tile([128, GW * SB], MMDT, tag="e")
                        nc.scalar.activation(e[:], sc[:],
                                             mybir.ActivationFunctionType.Exp,
                                             scale=SCALE)
                        d0 = GW * g - 4 * j
                        if d0 + GW > 0:  # group touches the causal diagonal
                            ev = e[:].rearrange("p (t f) -> p t f", t=GW)
                            nc.gpsimd.affine_select(
                                out=ev, in_=ev,
                                compare_op=mybir.AluOpType.is_ge,
                                fill=0.0, base=-128 * d0,
                                pattern=[[-128, GW], [1, SB]],
                                channel_multiplier=-1)
                        for t in range(GW):
                            i = GW * g + t
                            lhs = vaug[i][:].rearrange("p (h e) -> p h e", h=HG)[:, h, 0:DK + 1]
                            nc.tensor.matmul(
                                pv[0:DK + 1, :], lhs, e[:, SB * t:SB * t + SB],
                                start=(g == 0 and t == 0),
                                stop=(g == ngrp - 1 and t == GW - 1))
                    # normalize: ot = pv[0:64] * broadcast(1/pv[64])
                    rec = npool.tile([128, SB], MMDT, tag="rec")
                    with nc.allow_low_precision(reason="denominator recip in tf32"):
                        nc.vector.reciprocal(rec[DK:DK + 1, :], pv[DK:DK + 1, :])
                    bcp = bc_ps.tile([DK, SB], F32, tag="bc")
                    nc.tensor.matmul(bcp[:], onesc_sb[DK:DK + 1, :],
                                     rec[DK:DK + 1, :], start=True, stop=True)
                    bc = npool.tile([DK, SB], F32, tag="bcs")
                    nc.vector.tensor_copy(bc[:], bcp[:])
                    nc.vector.tensor_mul(ot[h][j][0:DK, :], pv[0:DK, :], bc[:])

        # ------------------------------------------------- output projection
        with tc.tile_pool(name="wo_ps", bufs=4, space="PSUM") as wo_ps:
            for st in range(NST):
                rq = slice(128 * (st % 4), 128 * (st % 4) + 128)
                jb = st // 4
                for dc in range(2):
                    cols = slice(SB * dc, SB * dc + SB)
                    ps = wo_ps.tile([128, SB], F32, tag="wo")
                    for h in range(HG):
                        nc.tensor.matmul(ps[:], ot[h][jb][:, rq], wo_sb[:, h, cols],
                                         start=(h == 0), stop=(h == HG - 1))
                    o_sb = opool.tile([128, SB], F32, tag="osb")
                    if (st + dc) % 2 == 0:
                        nc.vector.tensor_copy(o_sb[:], ps[:])
                    else:
                        nc.scalar.copy(o_sb[:], ps[:])
                    nc.sync.dma_start(out[128 * st:128 * st + 128, cols], o_sb[:])

    nc.compile()
    return nc


_CACHED_NC = {}


def _get_program(with_bias=False):
    if with_bias not in _CACHED_NC:
        _CACHED_NC[with_bias] = build_program(with_bias=with_bias)
    return _CACHED_NC[with_bias]


# ---------------------------------------------------------------------------
# entry point
# ---------------------------------------------------------------------------

def kernel(x, token_position, Wq, bq, Wk, bk, Wv, bv, Wo, bo, _results=None):
    from concourse.bass_utils import run_bass_kernel_spmd

    in_maps = make_core_inputs(x, token_position, Wq, bq, Wk, bk, Wv, bv, Wo, bo)
    if _results is None:
        with_bias = any(float(np.abs(np.asarray(v)).max()) != 0.0
                        for v in (bq, bk, bv))
        nc = _get_program(with_bias=with_bias)
        res = run_bass_kernel_spmd(nc, in_maps, list(range(N_CORES)))
        _results = [res.results[i]["out"] for i in range(N_CORES)]
    bo = np.asarray(bo, dtype=np.float32)
    out = np.empty((B, S, D), dtype=np.float32)
    for b in range(B):
        acc = _results[HG * b].astype(np.float32)
        for hg in range(1, HG):
            acc = acc + _results[HG * b + hg]
        out[b] = acc + bo[None, :]
    return out



# revision 7
# speedup vs baseline: 1.9880x; 1.9880x over previous
"""Trainium2 Bass kernel: causal multi-head attention with interleaved RoPE.

Problem shapes (hardcoded): x [2, 2048, 1024], 16 heads of dk=64.
Sharding: 8 cores = 2 batches x 4 head-groups (4 heads each). Each core
computes its head-slice Q/K/V projections, RoPE, causal attention, and a
partial output through its Wo row-slice; the host sums the 4 partials per
batch and adds bo.

Numerics: bf16 operands everywhere (validated ~5e-3 rel err vs the 2e-2
gate), fp32 PSUM accumulation.

Schedule: one flat instruction stream interleaving the three phases so the
PE never idles (idle gaps also drop the PE p-state from 2.4 to 1.2 GHz):
projection units of block j+1 and Wo units of block j-1 are issued as fill
between attention score/PV groups of block j. Causal masking is done by
accumulating a -200 upper-triangular tile into the diagonal 128x128 score
sub-blocks via identity-stationary matmuls (cheap, keeps the exp->PV chain
free of gpsimd). Softmax normalization runs entirely off the PE:
reciprocal_approx_fast (DVE) -> partition_broadcast (gpsimd) -> mul (DVE).
Wo packs head pairs on the contraction dim (128 rows fully used).

RoPE trick: attention scores are invariant to any permutation of the dk
axis applied to both Q and K, so the Wq/Wk columns are permuted on the host
into a "quadrant half-split" layout where each rotation pair partner sits
exactly 16 partitions away inside the same 32-partition quadrant. The DVE
stream_shuffle (a per-quadrant 32-way permute) then produces the swapped
operand, and RoPE becomes: rot = q * cosT + shuffle(q) * sinT with
host-precomputed tables (sinT carries the sign).
"""

from collections import deque
from contextlib import ExitStack

import numpy as np
import ml_dtypes

import concourse.bass as bass
import concourse.mybir as mybir
import concourse.tile as tile

B, S, D, H = 2, 2048, 1024, 16
DK = D // H  # 64
HG = 4  # heads per core
NCOLS = HG * DK  # 256 columns of the projection per core
THETA = 10000.0
SCALE = 1.0 / float(np.sqrt(DK))
N_CORES = 8

SB = 512            # sq block width
NSB = S // SB       # 4
NST = S // 128      # 16 key tiles / V tiles
NDC = D // 128      # 8 contraction chunks
GW = 2              # key tiles per score-psum group
AUGW = 72           # V head stride (65 used), 72*2B = 144 = 9*16B aligned

F32 = mybir.dt.float32
F32R = mybir.dt.float32r
BF16 = mybir.dt.bfloat16
BF = ml_dtypes.bfloat16
MASKVAL = -200.0


# ---------------------------------------------------------------------------
# host-side prep
# ---------------------------------------------------------------------------

def _rope_perm():
    """Within-head column permutation pi: new row r -> original dk index."""
    perm = np.empty(DK, dtype=np.int64)
    for r in range(DK):
        q, m = divmod(r, 32)
        if m < 16:
            perm[r] = 2 * (16 * q + m)
        else:
            perm[r] = 2 * (16 * q + m - 16) + 1
    return perm


_PERM = _rope_perm()
SHUF_MASK = list(range(16, 32)) + list(range(16))  # swap 16-halves per quadrant


def _rope_tables(pos):
    """cosT/sinT [128, S] fp32 for the permuted layout. pos: [S] int."""
    inv_freq = (np.float32(THETA) ** (-(np.arange(0, DK, 2, dtype=np.float32) / np.float32(DK))))  # [32]
    ang = pos.astype(np.float32)[:, None] * inv_freq[None, :]  # [S, 32]
    cos = np.cos(ang)
    sin = np.sin(ang)
    cosT = np.empty((128, S), dtype=np.float32)
    sinT = np.empty((128, S), dtype=np.float32)
    for p in range(128):
        r = p % DK
        q, m = divmod(r, 32)
        if m < 16:
            i = 16 * q + m
            sgn = -1.0
        else:
            i = 16 * q + m - 16
            sgn = 1.0
        cosT[p] = cos[:, i]
        sinT[p] = np.float32(sgn) * sin[:, i]
    return cosT, sinT


def bf16(a):
    return np.ascontiguousarray(np.asarray(a, dtype=np.float32).astype(BF))


def make_core_inputs(x, token_position, Wq, bq, Wk, bk, Wv, bv, Wo, bo):
    """Build the 8 per-core input maps."""
    x = np.asarray(x, dtype=np.float32)
    token_position = np.asarray(token_position)
    Wq, Wk, Wv, Wo = (np.asarray(w, dtype=np.float32) for w in (Wq, Wk, Wv, Wo))
    bq, bk, bv = (np.asarray(b_, dtype=np.float32) for b_ in (bq, bk, bv))

    # mask slabs for the 4 diagonal key-tile offsets d: key 128d+p masks
    # query q (tile-relative) iff 128d+p > q; -200 added into the score psum
    pp = np.arange(128)[:, None]
    qq = np.arange(SB)[None, :]
    maskslab = bf16(np.stack(
        [np.where(128 * dd + pp > qq, np.float32(MASKVAL), np.float32(0.0))
         for dd in range(4)], axis=1))  # [128, 4, 512]
    ident = bf16(np.eye(128, dtype=np.float32))
    onesr = np.ones((1, DK), dtype=np.float32)

    in_maps = []
    tables = {}
    for c in range(N_CORES):
        b, hg = divmod(c, HG)
        heads = range(HG * hg, HG * hg + HG)
        cols_qk = np.concatenate([DK * h + _PERM for h in heads])
        cols_v = np.arange(NCOLS * hg, NCOLS * hg + NCOLS)
        if b not in tables:
            tables[b] = _rope_tables(np.asarray(token_position[b]))
        cosT, sinT = tables[b]
        # wo rows packed as head pairs: [:, cpair, :] rows 0-63 head 2c, 64-127 head 2c+1
        wo_dev = np.stack([Wo[cols_v[128 * cp:128 * cp + 128], :] for cp in range(2)],
                          axis=1)  # [128, 2, 1024]
        in_maps.append({
            "xT": bf16(x[b].T),                       # [1024, 2048]
            "wq": bf16(Wq[:, cols_qk]),               # [1024, 256]
            "wk": bf16(Wk[:, cols_qk]),
            "wv": bf16(Wv[:, cols_v]),
            "wo": bf16(wo_dev),                       # [128, 2, 1024]
            "bq": bf16(bq[cols_qk][None, :]),         # [1, 256]
            "bk": bf16(bk[cols_qk][None, :]),
            "bv": bf16(bv[cols_v][None, :]),
            "ones_row": bf16(np.ones((1, SB), np.float32)),
            "onesr": onesr,                           # [1, 64] fp32 (f32r param)
            "ident": ident,                           # [128, 128]
            "maskslab": maskslab,                     # [128, 4, 512]
            "cosT": cosT,
            "sinT": sinT,
        })
    return in_maps


# ---------------------------------------------------------------------------
# device program
# ---------------------------------------------------------------------------

def build_program(with_bias=False):
    from concourse import bacc, library_config
    nc = bacc.Bacc("TRN2", debug=False)

    xT = nc.declare_dram_parameter("xT", [D, S], BF16, isOutput=False).ap()
    wq = nc.declare_dram_parameter("wq", [D, NCOLS], BF16, isOutput=False).ap()
    wk = nc.declare_dram_parameter("wk", [D, NCOLS], BF16, isOutput=False).ap()
    wv = nc.declare_dram_parameter("wv", [D, NCOLS], BF16, isOutput=False).ap()
    wo = nc.declare_dram_parameter("wo", [128, 2, D], BF16, isOutput=False).ap()
    bq = nc.declare_dram_parameter("bq", [1, NCOLS], BF16, isOutput=False).ap()
    bk = nc.declare_dram_parameter("bk", [1, NCOLS], BF16, isOutput=False).ap()
    bv = nc.declare_dram_parameter("bv", [1, NCOLS], BF16, isOutput=False).ap()
    ones_row_d = nc.declare_dram_parameter("ones_row", [1, SB], BF16, isOutput=False).ap()
    onesr_d = nc.declare_dram_parameter("onesr", [1, DK], F32R, isOutput=False).ap()
    ident_d = nc.declare_dram_parameter("ident", [128, 128], BF16, isOutput=False).ap()
    maskslab_d = nc.declare_dram_parameter("maskslab", [128, 4, SB], BF16, isOutput=False).ap()
    cosT = nc.declare_dram_parameter("cosT", [128, S], F32, isOutput=False).ap()
    sinT = nc.declare_dram_parameter("sinT", [128, S], F32, isOutput=False).ap()
    out = nc.declare_dram_parameter("out", [S, D], BF16, isOutput=True).ap()

    with tile.TileContext(nc) as tc, ExitStack() as ctx:
        nc.gpsimd.load_library(library_config.proxy)
        const = ctx.enter_context(tc.tile_pool(name="const", bufs=1))
        sbig = ctx.enter_context(tc.tile_pool(name="sbig", bufs=1))
        xts = ctx.enter_context(tc.tile_pool(name="xts", bufs=2))
        rtmp = ctx.enter_context(tc.tile_pool(name="rtmp", bufs=2))
        epool = ctx.enter_context(tc.tile_pool(name="epool", bufs=3))
        npool = ctx.enter_context(tc.tile_pool(name="npool", bufs=2))
        bpool = ctx.enter_context(tc.tile_pool(name="bpool", bufs=2))
        opool = ctx.enter_context(tc.tile_pool(name="opool", bufs=2))
        pj_ps = ctx.enter_context(tc.tile_pool(name="pj_ps", bufs=2, space="PSUM"))
        sc_ps = ctx.enter_context(tc.tile_pool(name="sc_ps", bufs=2, space="PSUM"))
        pv_ps = ctx.enter_context(tc.tile_pool(name="pv_ps", bufs=2, space="PSUM"))

        # --- DMA priority order: first projection's operands first
        wq_sb = [const.tile([128, NCOLS], BF16, tag=f"wq{dc}", name=f"wq{dc}")
                 for dc in range(NDC)]
        wk_sb = [const.tile([128, NCOLS], BF16, tag=f"wk{dc}", name=f"wk{dc}")
                 for dc in range(NDC)]
        wv_sb = [const.tile([128, NCOLS], BF16, tag=f"wv{dc}", name=f"wv{dc}")
                 for dc in range(NDC)]
        xt_tiles = {}

        def prefetch_xt(sb):
            ts = [xts.tile([128, SB], BF16, tag=f"xt{dc}", name=f"xt{sb}_{dc}")
                  for dc in range(NDC)]
            for dc in range(NDC):
                nc.sync.dma_start(ts[dc][:], xT[128 * dc:128 * dc + 128,
                                               SB * sb:SB * sb + SB])
            xt_tiles[sb] = ts

        for dc in range(NDC):
            nc.sync.dma_start(wq_sb[dc][:], wq[128 * dc:128 * dc + 128, :])
        prefetch_xt(0)
        for dc in range(NDC):
            nc.sync.dma_start(wk_sb[dc][:], wk[128 * dc:128 * dc + 128, :])
        cos_sb = const.tile([128, S], F32, tag="cos")
        sin_sb = const.tile([128, S], F32, tag="sin")
        nc.sync.dma_start(cos_sb[:, 0:SB], cosT[:, 0:SB])
        nc.sync.dma_start(sin_sb[:, 0:SB], sinT[:, 0:SB])
        for dc in range(NDC):
            nc.sync.dma_start(wv_sb[dc][:], wv[128 * dc:128 * dc + 128, :])
        nc.sync.dma_start(cos_sb[:, SB:], cosT[:, SB:])
        nc.sync.dma_start(sin_sb[:, SB:], sinT[:, SB:])
        onesr_sb = const.tile([1, DK], F32R, tag="onesr")
        nc.sync.dma_start(onesr_sb[:], onesr_d)
        ident_sb = const.tile([128, 128], BF16, tag="ident")
        nc.sync.dma_start(ident_sb[:], ident_d)
        maskslab_sb = const.tile([128, 4, SB], BF16, tag="maskslab")
        nc.sync.dma_start(maskslab_sb[:], maskslab_d)
        if with_bias:
            bq_sb = const.tile([1, NCOLS], BF16, tag="bq")
            bk_sb = const.tile([1, NCOLS], BF16, tag="bk")
            bv_sb = const.tile([1, NCOLS], BF16, tag="bv")
            ones_row = const.tile([1, SB], BF16, tag="ones_row")
            nc.sync.dma_start(bq_sb[:], bq)
            nc.sync.dma_start(bk_sb[:], bk)
            nc.sync.dma_start(bv_sb[:], bv)
            nc.sync.dma_start(ones_row[:], ones_row_d)
        wo_sb = const.tile([128, 2, D], BF16, tag="wo")
        nc.sync.dma_start(wo_sb[:], wo)

        # warm the Exp activation table during the projection phase
        scr = const.tile([1, 8], F32, tag="scr")
        nc.vector.memset(scr[:], 0.0)
        nc.scalar.activation(scr[:], scr[:], mybir.ActivationFunctionType.Exp)

        # --- persistent SBUF tensors
        qt = [[sbig.tile([128, SB], BF16, tag=f"qt{c}_{sb}", name=f"qt{c}_{sb}")
               for sb in range(NSB)] for c in range(2)]
        kth = [[sbig.tile([128, SB], BF16, tag=f"kh{h}_{sb}", name=f"kh{h}_{sb}")
                for sb in range(NSB)] for h in range(HG)]
        for h in range(HG):
            zrows = slice(DK, 128) if h % 2 == 0 else slice(0, DK)
            for sb in range(NSB):
                nc.vector.memset(kth[h][sb][zrows, :], 0.0)
        vaug = [sbig.tile([128, HG * AUGW], BF16, tag=f"va{st}", name=f"va{st}")
                for st in range(NST)]
        for st in range(NST):
            va = vaug[st][:].rearrange("p (h e) -> p h e", h=HG)
            nc.vector.memset(va[:, :, DK:DK + 1], 1.0)
        # normalized O^T head-pair tiles: rows 0-63 head 2c, 64-127 head 2c+1
        otp = [[sbig.tile([128, SB], BF16, tag=f"ot{c}_{j}", name=f"ot{c}_{j}")
                for j in range(NSB)] for c in range(2)]

        # ------------------------------------------------------ unit builders
        def proj_qk_unit(sb, c, kind):
            def run():
                ss = slice(SB * sb, SB * sb + SB)
                ncol = slice(128 * c, 128 * c + 128)
                w_sb = wq_sb if kind == "q" else wk_sb
                ps = pj_ps.tile([128, SB], F32, tag="pj")
                for dc in range(NDC):
                    nc.tensor.matmul(ps[:], w_sb[dc][:, ncol], xt_tiles[sb][dc][:],
                                     start=(dc == 0),
                                     stop=(dc == NDC - 1 and not with_bias))
                if with_bias:
                    b_sb = bq_sb if kind == "q" else bk_sb
                    nc.tensor.matmul(ps[:], b_sb[0:1, ncol], ones_row[0:1, :],
                                     start=False, stop=True)
                t_shuf = rtmp.tile([128, SB], F32, tag="rs")
                nc.vector.stream_shuffle(t_shuf[:], ps[:], SHUF_MASK)
                t_sin = rtmp.tile([128, SB], BF16, tag="rm")
                nc.gpsimd.tensor_mul(t_sin[:], t_shuf[:], sin_sb[:, ss])
                t_cos = rtmp.tile([128, SB], BF16, tag="rc")
                nc.vector.tensor_mul(t_cos[:], ps[:], cos_sb[:, ss])
                if kind == "q":
                    nc.vector.tensor_add(qt[c][sb][:], t_cos[:], t_sin[:])
                else:
                    nc.vector.tensor_add(kth[2 * c][sb][0:DK, :],
                                         t_cos[0:DK, :], t_sin[0:DK, :])
                    nc.vector.tensor_add(kth[2 * c + 1][sb][DK:128, :],
                                         t_cos[DK:128, :], t_sin[DK:128, :])
            return run

        def proj_v_unit(sb, st4):
            def run():
                st = 4 * sb + st4
                ps = pj_ps.tile([128, SB], F32, tag="pj")
                for dc in range(NDC):
                    nc.tensor.matmul(ps[:, 0:NCOLS],
                                     xt_tiles[sb][dc][:, 128 * st4:128 * st4 + 128],
                                     wv_sb[dc][:],
                                     start=(dc == 0),
                                     stop=(dc == NDC - 1 and not with_bias))
                if with_bias:
                    nc.tensor.matmul(ps[:, 0:NCOLS], ones_row[0:1, 0:128],
                                     bv_sb[0:1, :], start=False, stop=True)
                va = vaug[st][:].rearrange("p (h e) -> p h e", h=HG)
                nc.vector.tensor_copy(va[:, :, 0:DK],
                                      ps[:, 0:NCOLS].rearrange("p (h k) -> p h k", h=HG))
            return run

        def proj_units(sb):
            us = []
            for c in range(2):
                us.append(proj_qk_unit(sb, c, "q"))
                us.append(proj_qk_unit(sb, c, "k"))
            for st4 in range(4):
                us.append(proj_v_unit(sb, st4))
            return us

        def wo_unit(jb, st4):
            def run():
                st = 4 * jb + st4
                rq = slice(128 * st4, 128 * st4 + 128)
                o_sb = opool.tile([128, 2 * SB], BF16, tag="osb")
                for dc in range(2):
                    cols = slice(SB * dc, SB * dc + SB)
                    ps = pj_ps.tile([128, SB], F32, tag="pj")
                    for cp in range(2):
                        nc.tensor.matmul(ps[:], otp[cp][jb][:, rq],
                                         wo_sb[:, cp, cols],
                                         start=(cp == 0), stop=(cp == 1))
                    nc.vector.tensor_copy(o_sb[:, cols], ps[:])
                nc.sync.dma_start(out[128 * st:128 * st + 128, :], o_sb[:])
            return run

        # ------------------------------------------------------ attention
        pv_tiles = {}
        e_tiles = {}

        def sc_group(j, h, g):
            c = h // 2
            sc = sc_ps.tile([128, GW * SB], F32, tag="sc")
            gd = g - 2 * j  # >= 0 for diagonal groups
            for t in range(GW):
                i = GW * g + t
                nc.tensor.matmul(
                    sc[:, SB * t:SB * t + SB],
                    kth[h][i // 4][:, 128 * (i % 4):128 * (i % 4) + 128],
                    qt[c][j][:],
                    start=True, stop=(gd < 0),
                    skip_group_check=(gd >= 0))
            if gd >= 0:
                # -200 into every (key>query) position of each slab: full
                # columns left of the diagonal sub-block + its triangle
                for t in range(GW):
                    dd = 2 * gd + t
                    w = 128 * (dd + 1)
                    nc.tensor.matmul(sc[:, SB * t:SB * t + w], ident_sb[:],
                                     maskslab_sb[:, dd, 0:w],
                                     start=False, stop=(t == GW - 1),
                                     skip_group_check=True)
            e = epool.tile([128, GW * SB], BF16, tag="e")
            nc.scalar.activation(e[:], sc[:], mybir.ActivationFunctionType.Exp,
                                 scale=SCALE)
            e_tiles[(j, h, g)] = e

        def pv_group(j, h, g):
            def run():
                ngrp = 2 * (j + 1)
                if g == 0:
                    pv_tiles[(j, h)] = pv_ps.tile([DK + 1, SB], F32, tag="pv", name=f"pv{j}_{h}")
                pv = pv_tiles[(j, h)]
                e = e_tiles.pop((j, h, g))
                for t in range(GW):
                    i = GW * g + t
                    lhs = vaug[i][:].rearrange("p (h e) -> p h e", h=HG)[:, h, 0:DK + 1]
                    nc.tensor.matmul(pv[:], lhs, e[:, SB * t:SB * t + SB],
                                     start=(g == 0 and t == 0),
                                     stop=(g == ngrp - 1 and t == GW - 1))
            return run

        def norm_stages(j, h):
            cp, half = divmod(h, 2)
            box = {}

            def s1():  # reciprocal of the denominator row (ones-column of V)
                pv = pv_tiles[(j, h)]
                dn = npool.tile([1, SB], F32, tag="dn")
                nc.vector.tensor_copy(dn[:], pv[DK:DK + 1, :])
                rec = npool.tile([1, SB], F32, tag="rec")
                nc.vector.reciprocal_approx_fast(rec[:], dn[:])
                box["rec"] = rec

            def s2():  # broadcast 1/denom across the 64 dk partitions
                bc = bpool.tile([DK, SB], F32, tag="bc")
                nc.gpsimd.partition_broadcast(bc[:], box["rec"][:], channels=DK)
                box["bc"] = bc

            def s3():  # normalized O^T into the head-pair tile
                pv = pv_tiles.pop((j, h))
                nc.vector.tensor_mul(otp[cp][j][DK * half:DK * half + DK, :],
                                     pv[0:DK, :], box["bc"][:])
            return [s1, s2, s3]

        # ------------------------------------------------------ main schedule
        for u in proj_units(0):
            u()

        fills = deque()
        norm_q = deque()
        pending_pv = None
        for j in range(NSB):
            if j + 1 < NSB:
                fills.append(lambda sb=j + 1: prefetch_xt(sb))
                fills.extend(proj_units(j + 1))
            else:
                for jb in range(NSB - 1):
                    for st4 in range(4):
                        fills.append(wo_unit(jb, st4))
            nfill = len(fills)
            groups = [(h, g) for h in range(HG) for g in range(2 * (j + 1))]
            ng = len(groups)
            done = 0
            for idx, (h, g) in enumerate(groups):
                sc_group(j, h, g)
                want = (idx + 1) * nfill // ng
                while done < want and fills:
                    fills.popleft()()
                    done += 1
                if pending_pv is not None:
                    pending_pv()
                pending_pv = pv_group(j, h, g)
                if norm_q:
                    norm_q.popleft()()
                if g == 2 * (j + 1) - 1:
                    norm_q.extend(norm_stages(j, h))
            while fills:
                fills.popleft()()

        pending_pv()
        while norm_q:
            norm_q.popleft()()
        for st4 in range(4):
            wo_unit(NSB - 1, st4)()

    nc.compile()
    return nc


_CACHED_NC = {}


def _get_program(with_bias=False):
    if with_bias not in _CACHED_NC:
        _CACHED_NC[with_bias] = build_program(with_bias=with_bias)
    return _CACHED_NC[with_bias]


# ---------------------------------------------------------------------------
# entry point
# ---------------------------------------------------------------------------

def kernel(x, token_position, Wq, bq, Wk, bk, Wv, bv, Wo, bo, _results=None):
    from concourse.bass_utils import run_bass_kernel_spmd

    in_maps = make_core_inputs(x, token_position, Wq, bq, Wk, bk, Wv, bv, Wo, bo)
    if _results is None:
        with_bias = any(float(np.abs(np.asarray(v)).max()) != 0.0
                        for v in (bq, bk, bv))
        nc = _get_program(with_bias=with_bias)
        res = run_bass_kernel_spmd(nc, in_maps, list(range(N_CORES)))
        _results = [res.results[i]["out"] for i in range(N_CORES)]
    bo = np.asarray(bo, dtype=np.float32)
    out = np.empty((B, S, D), dtype=np.float32)
    for b in range(B):
        acc = np.asarray(_results[HG * b]).astype(np.float32)
        for hg in range(1, HG):
            acc = acc + np.asarray(_results[HG * b + hg]).astype(np.float32)
        out[b] = acc + bo[None, :]
    return out
